# revision 6
# baseline (speedup 1.0000x reference)
"""Linear (kernel-feature) attention for Trainium2, sharded over 8 NeuronCores.

Problem: B=4, H=16, S=4096, D=64 fp32.
    phi(x) = elu(x) + 1  (= exp(x) for x<=0, 1+x for x>0 = min(exp(x),1) + relu(x))
    kv   = phi_k_masked^T @ V          [d, v]
    k1   = sum_n phi_k_masked          [d]
    out  = (phi_q @ kv) / (phi_q @ k1 + eps)

Sharding: 64 (b,h) slices -> 8 per core (each core's slices share one batch b,
so one mask row per core). No cross-core communication.

Host-side layout (part of sharding, costs no HW time):
  - qT:  [4 pairs, 128, 4096]  = Q transposed per slice ([d, n]), two slices
         stacked on the partition dim. M2 contracts over d, so q must have d
         on partitions; transposing on host avoids any on-device transpose.
  - kc/vc/outc: [8 slices, 128, 32, 64] partition-tiled natural layout
         (row p holds n = t*128+p), giving 8KB contiguous DMA runs/partition.

Device pipeline per pair of slices:
  phi_k (ACT exp + DVE)  -> M1: 32 accumulating matmuls K=128 -> kv_ext[64,65]
  (two slices packed in PSUM partition halves via tile_position col-tiling)
  phi_q on transposed layout -> M2: per 128-row tile, kv_ext stationary?? no:
  lhsT = phi_qT tile [64d,128n] stationary, rhs = kv_ext[64,64]+k1[64,1]
  (two slices packed via row-tiling) -> PSUM [128n, 64v] + nrm [128,1]
  -> bulk reciprocal + fused divide on PSUM->SBUF evacuation -> store.
"""

import sys

sys.path.insert(0, "/opt/trn_rl_repo")

import numpy as np

B, H, S, D = 4, 16, 4096, 64
N_CORES = 8
SL = (B * H) // N_CORES  # slices per core = 8
PAIRS = SL // 2  # 4
NT = S // 128  # 32 n-tiles per slice
FREE = NT * D  # 2048 free cols for k/v/out slice layout
EPS = 1e-6

_programs: dict = {}


def _build_program(with_mask: bool, reps: int = 1):
    from contextlib import ExitStack

    import concourse.bacc as bacc
    import concourse.tile as tile
    from concourse import mybir

    f32 = mybir.dt.float32
    Alu = mybir.AluOpType
    Act = mybir.ActivationFunctionType

    nc = bacc.Bacc("TRN2", target_bir_lowering=False, debug=False)
    qT = nc.dram_tensor("qT", [PAIRS, 128, S], f32, kind="ExternalInput").ap()
    kc = nc.dram_tensor("kc", [SL, 128, FREE], f32, kind="ExternalInput").ap()
    vc = nc.dram_tensor("vc", [SL, 128, FREE], f32, kind="ExternalInput").ap()
    outc = nc.dram_tensor("outc", [SL, 128, FREE], f32, kind="ExternalOutput").ap()
    if with_mask:
        mpc = nc.dram_tensor("mask_pc", [128, NT], f32, kind="ExternalInput").ap()
        mfu = nc.dram_tensor("mask_full", [128, FREE], f32, kind="ExternalInput").ap()

    with tile.TileContext(nc) as tc, ExitStack() as ctx:
        singles = ctx.enter_context(tc.tile_pool(name="singles", bufs=1))
        kp = ctx.enter_context(tc.tile_pool(name="kp", bufs=3))
        vp = ctx.enter_context(tc.tile_pool(name="vp", bufs=3))
        qp = ctx.enter_context(tc.tile_pool(name="qp", bufs=2))
        tmp = ctx.enter_context(tc.tile_pool(name="tmp", bufs=3))
        kvp = ctx.enter_context(tc.tile_pool(name="kvp", bufs=2))
        nrmp = ctx.enter_context(tc.tile_pool(name="nrmp", bufs=4))
        outp = ctx.enter_context(tc.tile_pool(name="outp", bufs=2))
        ps_kv = ctx.enter_context(tc.tile_pool(name="ps_kv", bufs=2, space="PSUM"))
        ps_out = ctx.enter_context(tc.tile_pool(name="ps_out", bufs=3, space="PSUM"))
        ps_nrm = ctx.enter_context(tc.tile_pool(name="ps_nrm", bufs=2, space="PSUM"))

        ones_col = singles.tile([128, 1], f32)
        nc.vector.memset(ones_col, 1.0)
        if with_mask:
            mpc_sb = singles.tile([128, NT], f32)
            nc.sync.dma_start(out=mpc_sb, in_=mpc)
            mfu_sb = singles.tile([128, FREE], f32)
            nc.sync.dma_start(out=mfu_sb, in_=mfu)

        def phi_chunk(dst, src, scale):
            # dst = min(exp(scale*src),1) + scale*relu(src); dst may alias src
            e = tmp.tile([128, FREE], f32, tag="e")
            nc.scalar.activation(e, src, Act.Exp, scale=scale)
            r = tmp.tile([128, FREE], f32, tag="r")
            if scale == 1.0:
                nc.vector.tensor_scalar_max(r, src, 0.0)
            else:
                nc.vector.tensor_scalar(r, src, 0.0, scale, Alu.max, Alu.mult)
            nc.vector.scalar_tensor_tensor(dst, e, 1.0, r, Alu.min, Alu.add)

        for _rep in range(reps):
            for pair in range(PAIRS):
                s0 = 2 * pair
                # ---- K/V load + phi_k for the two slices of the pair
                phis, vts = [], []
                for r in range(2):
                    j = s0 + r
                    kt = kp.tile([128, FREE], f32)
                    nc.sync.dma_start(out=kt, in_=kc[j])
                    vt = vp.tile([128, FREE], f32)
                    nc.sync.dma_start(out=vt, in_=vc[j])
                    phi_chunk(kt, kt, 1.0)
                    if with_mask:
                        nc.vector.tensor_tensor(kt, kt, mfu_sb, Alu.mult)
                    phis.append(kt)
                    vts.append(vt)

                # ---- M1: kv_ext[64,65] per slice, packed into PSUM halves.
                # Only the first matmul touching each partition half uses
                # start=True (clears has_written bank-wide); the k1 column
                # then overwrites-on-first-touch and accumulates after.
                kv_ps = ps_kv.tile([128, 512], f32)
                for t in range(NT):
                    st, sp = (t == 0), (t == NT - 1)
                    red = mpc_sb[:, t : t + 1] if with_mask else ones_col[:, 0:1]
                    for r in range(2):
                        lhsT = phis[r][:, t * D : (t + 1) * D]
                        nc.tensor.matmul(
                            kv_ps[64 * r : 64 * r + 64, 0:64],
                            lhsT,
                            vts[r][:, t * D : (t + 1) * D],
                            start=st,
                            stop=sp,
                            tile_position=(0, 64 * r),
                            skip_group_check=True,
                        )
                        nc.tensor.matmul(
                            kv_ps[64 * r : 64 * r + 64, 64:65],
                            lhsT,
                            red,
                            start=False,
                            stop=sp,
                            tile_position=(0, 64 * r),
                            skip_group_check=True,
                        )
                kv_sb = kvp.tile([128, 65], f32)
                nc.vector.tensor_copy(kv_sb, kv_ps[:, 0:65])

                # ---- phi_q on transposed layout (two 2048-chunks share tmp)
                qt = qp.tile([128, S], f32)
                nc.sync.dma_start(out=qt, in_=qT[pair])
                for c in range(2):
                    sl = qt[:, c * FREE : (c + 1) * FREE]
                    phi_chunk(sl, sl, 0.125)

                # ---- M2 + divide + store per slice
                for r in range(2):
                    j = s0 + r
                    out_sb = outp.tile([128, FREE], f32)
                    rhs_kv = kv_sb[64 * r : 64 * r + 64, 0:64]
                    rhs_k1 = kv_sb[64 * r : 64 * r + 64, 64:65]
                    for g in range(NT // 8):
                        po = ps_out.tile([128, 512], f32)
                        pn = ps_nrm.tile([128, 512], f32)
                        for i in range(8):
                            t = g * 8 + i
                            lhsT = qt[64 * r : 64 * r + 64, t * 128 : (t + 1) * 128]
                            nc.tensor.matmul(
                                po[:, i * 64 : (i + 1) * 64],
                                lhsT,
                                rhs_kv,
                                start=(i == 0),
                                stop=(i == 7),
                                tile_position=(64 * r, 0),
                                skip_group_check=True,
                            )
                            nc.tensor.matmul(
                                pn[:, i : i + 1],
                                lhsT,
                                rhs_k1,
                                start=(i == 0),
                                stop=(i == 7),
                                tile_position=(64 * r, 0),
                                skip_group_check=True,
                            )
                        nsb = nrmp.tile([128, 8], f32)
                        nc.vector.tensor_scalar_add(nsb, pn[:, 0:8], EPS)
                        nc.vector.reciprocal(nsb, nsb)
                        for i in range(8):
                            t = g * 8 + i
                            nc.vector.tensor_scalar_mul(
                                out_sb[:, t * 64 : (t + 1) * 64],
                                po[:, i * 64 : (i + 1) * 64],
                                nsb[:, i : i + 1],
                            )
                    nc.sync.dma_start(out=outc[j], in_=out_sb)

    nc.compile()
    return nc


def _get_program(with_mask: bool, reps: int = 1):
    key = (with_mask, reps)
    if key not in _programs:
        _programs[key] = _build_program(with_mask, reps)
    return _programs[key]


def _pack_inputs(query, key, value, attention_mask):
    """Shard + lay out inputs for the 8 cores. Returns (in_maps, with_mask)."""
    q4 = np.asarray(query, dtype=np.float32).reshape(B * H, S, D)
    k4 = np.asarray(key, dtype=np.float32).reshape(B * H, S, D)
    v4 = np.asarray(value, dtype=np.float32).reshape(B * H, S, D)
    am = np.asarray(attention_mask, dtype=np.float32)

    # qT: [g, d, n] -> per core [PAIRS, 128, S]
    qT = np.ascontiguousarray(q4.transpose(0, 2, 1)).reshape(N_CORES, PAIRS, 128, S)
    # kc/vc: [g, t, p, d] -> [g, p, t, d]
    kcl = np.ascontiguousarray(
        k4.reshape(B * H, NT, 128, D).transpose(0, 2, 1, 3)
    ).reshape(N_CORES, SL, 128, FREE)
    vcl = np.ascontiguousarray(
        v4.reshape(B * H, NT, 128, D).transpose(0, 2, 1, 3)
    ).reshape(N_CORES, SL, 128, FREE)

    with_mask = not bool(np.all(am == 1.0))
    in_maps = []
    for c in range(N_CORES):
        m = {"qT": qT[c], "kc": kcl[c], "vc": vcl[c]}
        if with_mask:
            bc = (c * SL) // H  # all slices of a core share one batch row
            mp = np.ascontiguousarray(am[bc].reshape(NT, 128).T)  # [128, NT]
            m["mask_pc"] = mp
            m["mask_full"] = np.ascontiguousarray(
                np.repeat(mp[:, :, None], D, axis=2).reshape(128, FREE)
            )
        in_maps.append(m)
    return in_maps, with_mask


def _unpack_output(results):
    outs = np.stack([r["outc"] for r in results])  # [cores, SL, 128, FREE]
    outs = outs.reshape(B * H, 128, NT, D).transpose(0, 2, 1, 3)  # [g, t, p, d]
    return np.ascontiguousarray(outs).reshape(B, H, S, D)


def kernel(query, key, value, attention_mask):
    from concourse.bass_utils import run_bass_kernel_spmd

    in_maps, with_mask = _pack_inputs(query, key, value, attention_mask)
    nc = _get_program(with_mask)
    res = run_bass_kernel_spmd(nc, in_maps, core_ids=list(range(N_CORES)))
    return _unpack_output(res.results)


# revision 10
# speedup vs baseline: 333.2277x; 333.2277x over previous
"""Linear (kernel-feature) attention for Trainium2, sharded over 8 NeuronCores.

Problem: B=4, H=16, S=4096, D=64 fp32.
    phi(x) = elu(x) + 1  (= exp(x) for x<=0, 1+x for x>0 = min(exp(x),1) + relu(x))
    kv   = phi_k_masked^T @ V          [d, v]
    k1   = sum_n phi_k_masked          [d]
    out  = (phi_q @ kv) / (phi_q @ k1 + eps)

Sharding: 64 (b,h) slices -> 8 per core (each core's slices share one batch b,
so one mask row per core). No cross-core communication.

Host-side layout (part of sharding, costs no HW time):
  - qT:  [4 pairs, 128, 4096]  = Q transposed per slice ([d, n]), two slices
         stacked on the partition dim. M2 contracts over d, so q must have d
         on partitions; transposing on host avoids any on-device transpose.
  - kc/vc/outc: [8 slices, 128, 32, 64] partition-tiled natural layout
         (row p holds n = t*128+p), giving 8KB contiguous DMA runs/partition.

Device pipeline per pair of slices:
  phi_k (ACT exp + DVE)  -> M1: 32 accumulating matmuls K=128 -> kv_ext[64,65]
  (two slices packed in PSUM partition halves via tile_position col-tiling)
  phi_q on transposed layout -> M2: per 128-row tile, kv_ext stationary?? no:
  lhsT = phi_qT tile [64d,128n] stationary, rhs = kv_ext[64,64]+k1[64,1]
  (two slices packed via row-tiling) -> PSUM [128n, 64v] + nrm [128,1]
  -> bulk reciprocal + fused divide on PSUM->SBUF evacuation -> store.
"""

import sys

sys.path.insert(0, "/opt/trn_rl_repo")

import numpy as np

B, H, S, D = 4, 16, 4096, 64
N_CORES = 8
SL = (B * H) // N_CORES  # slices per core = 8
PAIRS = SL // 2  # 4
NT = S // 128  # 32 n-tiles per slice
FREE = NT * D  # 2048 free cols for k/v/out slice layout
EPS = 1e-6

_programs: dict = {}


def _build_program(with_mask: bool, reps: int = 1):
    from contextlib import ExitStack

    import concourse.bacc as bacc
    import concourse.tile as tile
    from concourse import mybir

    f32 = mybir.dt.float32
    Alu = mybir.AluOpType
    Act = mybir.ActivationFunctionType

    nc = bacc.Bacc("TRN2", target_bir_lowering=False, debug=False)
    qT = nc.dram_tensor("qT", [PAIRS, 128, S], f32, kind="ExternalInput").ap()
    kc = nc.dram_tensor("kc", [SL, 128, FREE], f32, kind="ExternalInput").ap()
    vc = nc.dram_tensor("vc", [SL, 128, FREE], f32, kind="ExternalInput").ap()
    outc = nc.dram_tensor("outc", [SL, 128, FREE], f32, kind="ExternalOutput").ap()
    if with_mask:
        mpc = nc.dram_tensor("mask_pc", [128, NT], f32, kind="ExternalInput").ap()
        mfu = nc.dram_tensor("mask_full", [128, FREE], f32, kind="ExternalInput").ap()

    with tile.TileContext(nc) as tc, ExitStack() as ctx:
        singles = ctx.enter_context(tc.tile_pool(name="singles", bufs=1))
        kp = ctx.enter_context(tc.tile_pool(name="kp", bufs=4))
        vp = ctx.enter_context(tc.tile_pool(name="vp", bufs=4))
        qp = ctx.enter_context(tc.tile_pool(name="qp", bufs=2))
        tmp = ctx.enter_context(tc.tile_pool(name="tmp", bufs=3))
        kvp = ctx.enter_context(tc.tile_pool(name="kvp", bufs=2))
        nrmp = ctx.enter_context(tc.tile_pool(name="nrmp", bufs=4))
        outp = ctx.enter_context(tc.tile_pool(name="outp", bufs=2))
        ps_kv = ctx.enter_context(tc.tile_pool(name="ps_kv", bufs=2, space="PSUM"))
        ps_out = ctx.enter_context(tc.tile_pool(name="ps_out", bufs=4, space="PSUM"))
        ps_nrm = ctx.enter_context(tc.tile_pool(name="ps_nrm", bufs=2, space="PSUM"))

        ones_col = singles.tile([128, 1], f32)
        nc.vector.memset(ones_col, 1.0)
        if with_mask:
            mpc_sb = singles.tile([128, NT], f32)
            nc.sync.dma_start(out=mpc_sb, in_=mpc)
            mfu_sb = singles.tile([128, FREE], f32)
            nc.sync.dma_start(out=mfu_sb, in_=mfu)

        def phi_chunk(dst, src, scale):
            # dst = min(exp(scale*src),1) + scale*relu(src); dst may alias src.
            # Exp and Relu share one ACT table (exp_and_others) -> no switch.
            e = tmp.tile([128, FREE], f32, tag="e")
            nc.scalar.activation(e, src, Act.Exp, scale=scale)
            r = tmp.tile([128, FREE], f32, tag="r")
            nc.scalar.activation(r, src, Act.Relu, scale=scale)
            nc.vector.scalar_tensor_tensor(dst, e, 1.0, r, Alu.min, Alu.add)

        for _rep in range(reps):
            for pair in range(PAIRS):
                s0 = 2 * pair
                # ---- K/V load + phi_k for the two slices of the pair
                phis, vts = [], []
                for r in range(2):
                    j = s0 + r
                    kt = kp.tile([128, FREE], f32)
                    nc.sync.dma_start(out=kt, in_=kc[j])
                    vt = vp.tile([128, FREE], f32)
                    nc.sync.dma_start(out=vt, in_=vc[j])
                    phi_chunk(kt, kt, 1.0)
                    if with_mask:
                        nc.vector.tensor_tensor(kt, kt, mfu_sb, Alu.mult)
                    phis.append(kt)
                    vts.append(vt)

                # ---- M1: kv_ext[64,65] per slice, packed into PSUM halves.
                # Only the first matmul touching each partition half uses
                # start=True (clears has_written bank-wide); the k1 column
                # then overwrites-on-first-touch and accumulates after.
                kv_ps = ps_kv.tile([128, 512], f32)
                for t in range(NT):
                    st, sp = (t == 0), (t == NT - 1)
                    red = mpc_sb[:, t : t + 1] if with_mask else ones_col[:, 0:1]
                    for r in range(2):
                        lhsT = phis[r][:, t * D : (t + 1) * D]
                        nc.tensor.matmul(
                            kv_ps[64 * r : 64 * r + 64, 0:64],
                            lhsT,
                            vts[r][:, t * D : (t + 1) * D],
                            start=st,
                            stop=sp,
                            tile_position=(0, 64 * r),
                            skip_group_check=True,
                        )
                        nc.tensor.matmul(
                            kv_ps[64 * r : 64 * r + 64, 64:65],
                            lhsT,
                            red,
                            start=False,
                            stop=sp,
                            tile_position=(0, 64 * r),
                            skip_group_check=True,
                        )
                kv_sb = kvp.tile([128, 65], f32)
                nc.vector.tensor_copy(kv_sb, kv_ps[:, 0:65])

                # ---- phi_q on transposed layout (two 2048-chunks share tmp)
                qt = qp.tile([128, S], f32)
                nc.sync.dma_start(out=qt, in_=qT[pair])
                for c in range(2):
                    sl = qt[:, c * FREE : (c + 1) * FREE]
                    phi_chunk(sl, sl, 0.125)

                # ---- M2 + divide + store per slice
                for r in range(2):
                    j = s0 + r
                    out_sb = outp.tile([128, FREE], f32)
                    rhs_kv = kv_sb[64 * r : 64 * r + 64, 0:64]
                    rhs_k1 = kv_sb[64 * r : 64 * r + 64, 64:65]
                    for g in range(NT // 8):
                        po = ps_out.tile([128, 512], f32)
                        pn = ps_nrm.tile([128, 512], f32)
                        for i in range(8):
                            t = g * 8 + i
                            lhsT = qt[64 * r : 64 * r + 64, t * 128 : (t + 1) * 128]
                            nc.tensor.matmul(
                                po[:, i * 64 : (i + 1) * 64],
                                lhsT,
                                rhs_kv,
                                start=(i == 0),
                                stop=(i == 7),
                                tile_position=(64 * r, 0),
                                skip_group_check=True,
                            )
                            nc.tensor.matmul(
                                pn[:, i : i + 1],
                                lhsT,
                                rhs_k1,
                                start=(i == 0),
                                stop=(i == 7),
                                tile_position=(64 * r, 0),
                                skip_group_check=True,
                            )
                        nsb = nrmp.tile([128, 8], f32)
                        nc.vector.tensor_scalar_add(nsb, pn[:, 0:8], EPS)
                        nc.vector.reciprocal(nsb, nsb)
                        # one fused divide for the whole bank: recip column
                        # broadcast along v via a step-0 AP
                        nc.vector.tensor_tensor(
                            out_sb[:, g * 512 : (g + 1) * 512].rearrange(
                                "p (a b) -> p a b", a=8
                            ),
                            po[:, :].rearrange("p (a b) -> p a b", a=8),
                            nsb.broadcast_to([128, 8, 64]),
                            Alu.mult,
                        )
                    nc.sync.dma_start(out=outc[j], in_=out_sb)

    nc.compile()
    return nc


def _get_program(with_mask: bool, reps: int = 1):
    key = (with_mask, reps)
    if key not in _programs:
        _programs[key] = _build_program(with_mask, reps)
    return _programs[key]


def _pack_inputs(query, key, value, attention_mask):
    """Shard + lay out inputs for the 8 cores. Returns (in_maps, with_mask)."""
    q4 = np.asarray(query, dtype=np.float32).reshape(B * H, S, D)
    k4 = np.asarray(key, dtype=np.float32).reshape(B * H, S, D)
    v4 = np.asarray(value, dtype=np.float32).reshape(B * H, S, D)
    am = np.asarray(attention_mask, dtype=np.float32)

    # qT: [g, d, n] -> per core [PAIRS, 128, S]
    qT = np.ascontiguousarray(q4.transpose(0, 2, 1)).reshape(N_CORES, PAIRS, 128, S)
    # kc/vc: [g, t, p, d] -> [g, p, t, d]
    kcl = np.ascontiguousarray(
        k4.reshape(B * H, NT, 128, D).transpose(0, 2, 1, 3)
    ).reshape(N_CORES, SL, 128, FREE)
    vcl = np.ascontiguousarray(
        v4.reshape(B * H, NT, 128, D).transpose(0, 2, 1, 3)
    ).reshape(N_CORES, SL, 128, FREE)

    with_mask = not bool(np.all(am == 1.0))
    in_maps = []
    for c in range(N_CORES):
        m = {"qT": qT[c], "kc": kcl[c], "vc": vcl[c]}
        if with_mask:
            bc = (c * SL) // H  # all slices of a core share one batch row
            mp = np.ascontiguousarray(am[bc].reshape(NT, 128).T)  # [128, NT]
            m["mask_pc"] = mp
            m["mask_full"] = np.ascontiguousarray(
                np.repeat(mp[:, :, None], D, axis=2).reshape(128, FREE)
            )
        in_maps.append(m)
    return in_maps, with_mask


def _unpack_output(results):
    outs = np.stack([r["outc"] for r in results])  # [cores, SL, 128, FREE]
    outs = outs.reshape(B * H, 128, NT, D).transpose(0, 2, 1, 3)  # [g, t, p, d]
    return np.ascontiguousarray(outs).reshape(B, H, S, D)


def kernel(query, key, value, attention_mask):
    from concourse.bass_utils import run_bass_kernel_spmd

    in_maps, with_mask = _pack_inputs(query, key, value, attention_mask)
    nc = _get_program(with_mask)
    res = run_bass_kernel_spmd(nc, in_maps, core_ids=list(range(N_CORES)))
    return _unpack_output(res.results)
